# revision 10
# baseline (speedup 1.0000x reference)
"""GQA attention forward (B=2, S=2048, D=2048, 16 q heads / 4 kv heads, RoPE,
causal) on 8 Trainium2 NeuronCores.

Sharding: core c <-> (batch b = c//4, kv-group g = c%4). Each core computes its
4 query heads + 1 kv head end-to-end, including its row-shard of wo; the host
sums the 4 wo-partials per batch (the "all-reduce after wo" of the tensor
parallel scheme, done at gather time).

Layout tricks:
  - x is passed transposed (d-major) so every matmul contraction dim lands on
    SBUF partitions.
  - wq/wk columns are permuted per head (even dims -> partitions 0..63, odd ->
    64..127) so RoPE becomes plain elementwise DVE math on partition halves.
    The permutation cancels in q.k dot products.
  - all matmuls run in bf16 (2x the fp32r streaming rate on HW); accumulation
    stays fp32 in PSUM. End-to-end rel err ~4e-3 (validated on host).
  - scores are built transposed ([t, s]); the softmax denominator is an
    all-ones-matrix matmul accumulated in PSUM, which lands the denominator
    already broadcast across partitions (no gpsimd broadcast needed).
  - the attention inner loop is software-pipelined (scores/exp run a few
    tiles ahead of the AV/denominator matmuls) so the PE never waits on the
    scalar-engine exp.
  - attention output stays in SBUF; the wo matmuls run per s-block right
    after that block's attention, and y is written out in bf16 (the host
    sums the 4 per-core partials in fp32).
"""

import ml_dtypes
import numpy as np

BF = ml_dtypes.bfloat16
B, S, D = 2, 2048, 2048
N_HEADS, N_KV_HEADS, HD = 16, 4, 128
NH = N_HEADS // N_KV_HEADS  # q heads per core = 4
SB = 512                    # s-block (moving dim per matmul)
NSJ = S // SB               # 4 s-blocks
NT = S // HD                # 16 t-tiles (and d-tiles)
NM = NH + 2                 # 6 projection column-blocks: k, v, q0..q3
H2 = HD // 2
SCALE = 1.0 / np.sqrt(HD).astype(np.float32)

_PROG = None  # built once per process


def _build_program():
    import concourse.bacc as bacc
    import concourse.tile as tile
    from concourse import mybir

    F32 = mybir.dt.float32
    BF16 = mybir.dt.bfloat16
    Exp = mybir.ActivationFunctionType.Exp

    nc = bacc.Bacc("TRN2", target_bir_lowering=False, debug=False)

    xt_d = nc.declare_dram_parameter("xt", [D, S], BF16, isOutput=False)
    wqkv_d = nc.declare_dram_parameter("wqkv", [D, NM * HD], BF16, isOutput=False)
    wo_d = nc.declare_dram_parameter("wo", [NH * HD, D], BF16, isOutput=False)
    cost_d = nc.declare_dram_parameter("cost", [H2, S], F32, isOutput=False)
    sint_d = nc.declare_dram_parameter("sint", [H2, S], F32, isOutput=False)
    tri_d = nc.declare_dram_parameter("tri", [HD, HD], BF16, isOutput=False)
    ident_d = nc.declare_dram_parameter("ident", [HD, HD], BF16, isOutput=False)
    ones_d = nc.declare_dram_parameter("ones", [HD, HD], BF16, isOutput=False)
    y_d = nc.declare_dram_parameter("y", [S, D], BF16, isOutput=True)

    with tile.TileContext(nc) as tc:
        with (
            tc.tile_pool(name="consts", bufs=1) as consts,
            tc.tile_pool(name="persist", bufs=1) as persist,
            tc.tile_pool(name="work", bufs=2) as work,
            tc.tile_pool(name="xts_pool", bufs=1) as xts_pool,
            tc.tile_pool(name="qk_pool", bufs=1) as qk_pool,
            tc.tile_pool(name="es_pool", bufs=1) as es_pool,
            tc.tile_pool(name="ps", bufs=1, space="PSUM") as ps,
        ):
            tri = consts.tile([HD, HD], BF16, tag="tri")
            ident = consts.tile([HD, HD], BF16, tag="ident")
            ones_sb = consts.tile([HD, HD], BF16, tag="ones")
            cost = consts.tile([H2, S], F32, tag="cost")
            sint = consts.tile([H2, S], F32, tag="sint")

            wqkv = persist.tile([HD, NT, NM * HD], BF16, tag="wqkv")
            kt = persist.tile([HD, S], BF16, tag="kt")
            v_sb = persist.tile([HD, NT, HD], BF16, tag="v_sb")
            on_sb = persist.tile([HD, NH, S], BF16, tag="on")
            wo_sb = persist.tile([HD, NH, D], BF16, tag="wo")

            xt_r = xt_d[:, :].rearrange("(t p) s -> p t s", p=HD)
            wqkv_r = wqkv_d[:, :].rearrange("(t p) m -> p t m", p=HD)

            # startup DMA order matters: cost/sint lead the scalar HW queue so
            # rope never waits behind the 3MB wqkv; all 4 x blocks prefetch up
            # front on the sync HW queue (y writes ride the same queue later).
            nc.scalar.dma_start(out=cost, in_=cost_d[:, :])
            nc.scalar.dma_start(out=sint, in_=sint_d[:, :])
            xts_tiles = {}
            for ck in range(4):
                nc.scalar.dma_start(
                    out=wqkv[:, ck * 4 : (ck + 1) * 4, :],
                    in_=wqkv_r[:, ck * 4 : (ck + 1) * 4, :],
                )
                xq = xts_pool.tile(
                    [HD, NT // 4, SB], BF16, tag="xts", bufs=16, name=f"xts_0_{ck}"
                )
                nc.sync.dma_start(out=xq, in_=xt_r[:, ck * 4 : (ck + 1) * 4, 0:SB])
                xts_tiles[(0, ck)] = xq
            for sj in range(1, NSJ):
                for ck in range(4):
                    xq = xts_pool.tile(
                        [HD, NT // 4, SB], BF16, tag="xts", bufs=16,
                        name=f"xts_{sj}_{ck}",
                    )
                    nc.sync.dma_start(
                        out=xq,
                        in_=xt_r[:, ck * 4 : (ck + 1) * 4, sj * SB : (sj + 1) * SB],
                    )
                    xts_tiles[(sj, ck)] = xq
            nc.gpsimd.dma_start(out=tri, in_=tri_d[:, :])
            nc.gpsimd.dma_start(out=ident, in_=ident_d[:, :])
            nc.gpsimd.dma_start(out=ones_sb, in_=ones_d[:, :])
            nc.scalar.dma_start(
                out=wo_sb, in_=wo_d[:, :].rearrange("(h p) d -> p h d", p=HD)
            )

            for sj in range(NSJ):
                s0 = sj * SB
                quarters = [xts_tiles[(sj, ck)] for ck in range(4)]

                # ---- projection of x^T[:, s-block]: k, v, q0..q3 ----
                q_tiles = []
                for m in range(NM):
                    pp = ps.tile([HD, SB], F32, tag="pp", bufs=2, name=f"pp_{sj}_{m}")
                    for dt in range(NT):
                        nc.tensor.matmul(
                            out=pp,
                            lhsT=wqkv[:, dt, m * HD : (m + 1) * HD],
                            rhs=quarters[dt // 4][:, dt % 4, :],
                            start=(dt == 0),
                            stop=(dt == NT - 1),
                        )
                    if m == 1:
                        # v: psum holds v^T[hd, s-block]; transpose to v[t, hd]
                        vt = work.tile([HD, SB], BF16, tag="vt")
                        nc.scalar.copy(out=vt, in_=pp)
                        for qq in range(SB // HD):
                            pt = ps.tile(
                                [HD, HD], BF16, tag="pp", bufs=2, name=f"pt_{sj}_{qq}"
                            )
                            nc.tensor.transpose(pt, vt[:, qq * HD : (qq + 1) * HD], ident)
                            nc.scalar.copy(out=v_sb[:, sj * 4 + qq, :], in_=pt)
                    else:
                        # rope: rows 0:64 = even dims (xr), 64:128 = odd (xi)
                        # out_even = xr*c - xi*s ; out_odd = xr*s + xi*c
                        # muls on DVE (PSUM input side-steps the same-base-
                        # partition rule); combines on gpsimd (same-base SB)
                        if m == 0:
                            dst = kt[:, s0 : s0 + SB]
                        else:
                            dst = qk_pool.tile(
                                [HD, SB], BF16, tag="qk", bufs=8, name=f"q_{sj}_{m}"
                            )
                            q_tiles.append(dst)
                        c = cost[:, s0 : s0 + SB]
                        sn = sint[:, s0 : s0 + SB]
                        ta = work.tile([H2, SB], F32, tag="ropeA")
                        tb = work.tile([H2, SB], F32, tag="ropeB")
                        nc.vector.tensor_mul(out=ta, in0=pp[0:H2, :], in1=c)
                        nc.vector.tensor_mul(out=tb, in0=pp[H2:HD, :], in1=sn)
                        nc.gpsimd.tensor_sub(out=dst[0:H2, :], in0=ta, in1=tb)
                        tc2 = work.tile([H2, SB], F32, tag="ropeA")
                        td = work.tile([H2, SB], F32, tag="ropeB")
                        nc.vector.tensor_mul(out=tc2, in0=pp[0:H2, :], in1=sn)
                        nc.vector.tensor_mul(out=td, in0=pp[H2:HD, :], in1=c)
                        nc.gpsimd.tensor_add(out=dst[H2:HD, :], in0=tc2, in1=td)

                # ---- attention, software-pipelined over t-tiles ----
                nt = 4 * sj + 4  # causal: t-tiles 0..nt-1
                LOOKAHEAD = 3
                for h in range(NH):
                    qts = q_tiles[h]
                    ps_o = ps.tile([HD, SB], F32, tag="o", bufs=2, name=f"o_{sj}_{h}")
                    ps_den = ps.tile(
                        [HD, SB], F32, tag="den", bufs=1, name=f"den_{sj}_{h}"
                    )

                    def emit_front(ti):
                        kdiag = ti - 4 * sj
                        c0 = max(0, kdiag) * HD  # first valid column (diag band)
                        ps_s = ps.tile(
                            [HD, SB], F32, tag="s", bufs=3, name=f"s_{sj}_{h}_{ti}"
                        )
                        nc.tensor.matmul(
                            out=ps_s[:, c0:SB],
                            lhsT=kt[:, ti * HD : (ti + 1) * HD],
                            rhs=qts[:, c0:SB],
                            start=True,
                            stop=True,
                        )
                        es = es_pool.tile(
                            [HD, SB], BF16, tag="es", bufs=6, name=f"es_{sj}_{h}_{ti}"
                        )
                        nc.scalar.activation(
                            out=es[:, c0:SB], in_=ps_s[:, c0:SB], func=Exp,
                            scale=float(SCALE),
                        )
                        if kdiag >= 0:
                            # triangular part: first HD valid columns
                            nc.gpsimd.tensor_mul(
                                out=es[:, c0 : c0 + HD],
                                in0=es[:, c0 : c0 + HD],
                                in1=tri,
                            )
                        return (ti, es, c0)

                    def emit_back(item):
                        ti, es, c0 = item
                        nc.tensor.matmul(
                            out=ps_o[:, c0:SB],
                            lhsT=v_sb[:, ti, :],
                            rhs=es[:, c0:SB],
                            start=(ti == 0),
                            stop=(ti == nt - 1),
                        )
                        nc.tensor.matmul(
                            out=ps_den[:, c0:SB],
                            lhsT=ones_sb,
                            rhs=es[:, c0:SB],
                            start=(ti == 0),
                            stop=(ti == nt - 1),
                        )

                    pend = []
                    for ti in range(nt):
                        pend.append(emit_front(ti))
                        if len(pend) > LOOKAHEAD:
                            emit_back(pend.pop(0))
                    while pend:
                        emit_back(pend.pop(0))

                    # normalize: on = ps_o * (1/den); den is already broadcast
                    rb = work.tile([HD, SB], F32, tag="rb")
                    nc.vector.reciprocal_approx_fast(out=rb, in_=ps_den)
                    nc.vector.tensor_mul(
                        out=on_sb[:, h, s0 : s0 + SB], in0=ps_o, in1=rb
                    )

                # ---- wo for this s-block's t-tiles ----
                for stl in range(4):
                    st = sj * 4 + stl
                    t0 = st * HD
                    y_row = work.tile([HD, D], BF16, tag="ysb", bufs=2, name=f"yr_{st}")
                    for dj in range(NSJ):
                        ps_y = ps.tile(
                            [HD, SB], F32, tag="pp", bufs=2, name=f"ps_y_{st}_{dj}"
                        )
                        for hh in range(NH):
                            nc.tensor.matmul(
                                out=ps_y,
                                lhsT=on_sb[:, hh, t0 : t0 + HD],
                                rhs=wo_sb[:, hh, dj * SB : (dj + 1) * SB],
                                start=(hh == 0),
                                stop=(hh == NH - 1),
                            )
                        nc.vector.tensor_copy(y_row[:, dj * SB : (dj + 1) * SB], ps_y)
                    nc.sync.dma_start(out=y_d[t0 : t0 + HD, :], in_=y_row)

    nc.compile()
    return nc


def _get_program():
    global _PROG
    if _PROG is None:
        _PROG = _build_program()
    return _PROG


def _make_in_maps(x, freqs_cos, freqs_sin, wq, wk, wv, wo):
    perm = np.concatenate([np.arange(0, HD, 2), np.arange(1, HD, 2)])  # even|odd

    costT = np.ascontiguousarray(np.asarray(freqs_cos, np.float32).T)  # [64, S]
    sintT = np.ascontiguousarray(np.asarray(freqs_sin, np.float32).T)

    tt = np.arange(HD)[:, None]
    ss = np.arange(HD)[None, :]
    tri = (tt <= ss).astype(BF)  # lower-tri in [t, s]: valid iff t <= s
    ident = np.eye(HD, dtype=BF)
    ones = np.ones((HD, HD), dtype=BF)

    # permute q/k head-dim columns so rope pairs land on partition halves
    def permute_heads(w, n_heads):
        w = np.asarray(w, np.float32).reshape(D, n_heads, HD)
        return w[:, :, perm].reshape(D, n_heads * HD)

    wq_p = permute_heads(wq, N_HEADS)
    wk_p = permute_heads(wk, N_KV_HEADS)
    wv_ = np.asarray(wv, np.float32)
    wo_ = np.asarray(wo, np.float32)
    x_ = np.asarray(x, np.float32)

    in_maps = []
    for c in range(8):
        b, g = divmod(c, 4)
        wqkv = np.concatenate(
            [
                wk_p[:, g * HD : (g + 1) * HD],
                wv_[:, g * HD : (g + 1) * HD],
                wq_p[:, g * NH * HD : (g + 1) * NH * HD],
            ],
            axis=1,
        )
        in_maps.append(
            {
                "xt": np.ascontiguousarray(x_[b].T).astype(BF),
                "wqkv": np.ascontiguousarray(wqkv).astype(BF),
                "wo": np.ascontiguousarray(
                    wo_[g * NH * HD : (g + 1) * NH * HD, :]
                ).astype(BF),
                "cost": costT,
                "sint": sintT,
                "tri": tri,
                "ident": ident,
                "ones": ones,
            }
        )
    return in_maps


def run(x, freqs_cos, freqs_sin, wq, wk, wv, wo, trace=False):
    from concourse.bass_utils import run_bass_kernel_spmd

    nc = _get_program()
    in_maps = _make_in_maps(x, freqs_cos, freqs_sin, wq, wk, wv, wo)
    res = run_bass_kernel_spmd(nc, in_maps, list(range(8)), trace=trace)
    out = np.empty((B, S, D), dtype=np.float32)
    for b in range(B):
        acc = res.results[b * 4]["y"].astype(np.float32)
        for g in range(1, 4):
            acc = acc + res.results[b * 4 + g]["y"].astype(np.float32)
        out[b] = acc
    return out, res


def kernel(x, freqs_cos, freqs_sin, wq, wk, wv, wo):
    out, _ = run(x, freqs_cos, freqs_sin, wq, wk, wv, wo, trace=False)
    return out


# revision 14
# speedup vs baseline: 1.0254x; 1.0254x over previous
"""GQA attention forward (B=2, S=2048, D=2048, 16 q heads / 4 kv heads, RoPE,
causal) on 8 Trainium2 NeuronCores.

Sharding: core c <-> (batch b = c//4, kv-group g = c%4). Each core computes its
4 query heads + 1 kv head end-to-end, including its row-shard of wo; the host
sums the 4 wo-partials per batch (the "all-reduce after wo" of the tensor
parallel scheme, done at gather time).

Layout tricks:
  - x is passed transposed (d-major) so every matmul contraction dim lands on
    SBUF partitions.
  - wq/wk columns are permuted per head (even dims -> partitions 0..63, odd ->
    64..127) so RoPE becomes plain elementwise DVE math on partition halves.
    The permutation cancels in q.k dot products.
  - all matmuls run in bf16 (2x the fp32r streaming rate on HW); accumulation
    stays fp32 in PSUM. End-to-end rel err ~4e-3 (validated on host).
  - scores are built transposed ([t, s]); the softmax denominator is an
    all-ones-matrix matmul accumulated in PSUM, which lands the denominator
    already broadcast across partitions (no gpsimd broadcast needed).
  - the attention inner loop is software-pipelined (scores/exp run a few
    tiles ahead of the AV/denominator matmuls) so the PE never waits on the
    scalar-engine exp.
  - attention output stays in SBUF; the wo matmuls run per s-block right
    after that block's attention, and y is written out in bf16 (the host
    sums the 4 per-core partials in fp32).
"""

import ml_dtypes
import numpy as np

BF = ml_dtypes.bfloat16
B, S, D = 2, 2048, 2048
N_HEADS, N_KV_HEADS, HD = 16, 4, 128
NH = N_HEADS // N_KV_HEADS  # q heads per core = 4
SB = 512                    # s-block (moving dim per matmul)
NSJ = S // SB               # 4 s-blocks
NT = S // HD                # 16 t-tiles (and d-tiles)
NM = NH + 2                 # 6 projection column-blocks: k, v, q0..q3
H2 = HD // 2
SCALE = 1.0 / np.sqrt(HD).astype(np.float32)

_PROG = None  # built once per process


def _build_program():
    import concourse.bacc as bacc
    import concourse.tile as tile
    from concourse import mybir

    F32 = mybir.dt.float32
    BF16 = mybir.dt.bfloat16
    Exp = mybir.ActivationFunctionType.Exp

    nc = bacc.Bacc("TRN2", target_bir_lowering=False, debug=False)

    xt_d = nc.declare_dram_parameter("xt", [D, S], BF16, isOutput=False)
    wqkv_d = nc.declare_dram_parameter("wqkv", [D, NM * HD], BF16, isOutput=False)
    wo_d = nc.declare_dram_parameter("wo", [NH * HD, D], BF16, isOutput=False)
    cost_d = nc.declare_dram_parameter("cost", [H2, S], F32, isOutput=False)
    sint_d = nc.declare_dram_parameter("sint", [H2, S], F32, isOutput=False)
    tri_d = nc.declare_dram_parameter("tri", [HD, HD], BF16, isOutput=False)
    ident_d = nc.declare_dram_parameter("ident", [HD, HD], BF16, isOutput=False)
    ones_d = nc.declare_dram_parameter("ones", [HD, HD], BF16, isOutput=False)
    y_d = nc.declare_dram_parameter("y", [S, D], BF16, isOutput=True)

    with tile.TileContext(nc) as tc:
        with (
            tc.tile_pool(name="consts", bufs=1) as consts,
            tc.tile_pool(name="persist", bufs=1) as persist,
            tc.tile_pool(name="work", bufs=2) as work,
            tc.tile_pool(name="xts_pool", bufs=1) as xts_pool,
            tc.tile_pool(name="qk_pool", bufs=1) as qk_pool,
            tc.tile_pool(name="es_pool", bufs=1) as es_pool,
            tc.tile_pool(name="ps", bufs=1, space="PSUM") as ps,
        ):
            tri = consts.tile([HD, HD], BF16, tag="tri")
            ident = consts.tile([HD, HD], BF16, tag="ident")
            ones_sb = consts.tile([HD, HD], BF16, tag="ones")
            cost = consts.tile([H2, S], F32, tag="cost")
            sint = consts.tile([H2, S], F32, tag="sint")

            wqkv = persist.tile([HD, NT, NM * HD], BF16, tag="wqkv")
            kt = persist.tile([HD, S], BF16, tag="kt")
            v_sb = persist.tile([HD, NT, HD], BF16, tag="v_sb")
            on_sb = persist.tile([HD, NH, S], BF16, tag="on")
            wo_sb = persist.tile([HD, NH, D], BF16, tag="wo")

            xt_r = xt_d[:, :].rearrange("(t p) s -> p t s", p=HD)
            wqkv_r = wqkv_d[:, :].rearrange("(t p) m -> p t m", p=HD)

            # startup DMA order matters: cost/sint lead the scalar HW queue so
            # rope never waits behind the 3MB wqkv; x block 0 loads as quarters
            # on the sync HW queue, blocks 1-3 as one wide (3KB-line) load per
            # dt-quarter (y writes ride the sync queue later).
            nc.scalar.dma_start(out=cost, in_=cost_d[:, :])
            nc.scalar.dma_start(out=sint, in_=sint_d[:, :])
            xts_tiles = {}
            xrest_tiles = {}
            for ck in range(4):
                nc.scalar.dma_start(
                    out=wqkv[:, ck * 4 : (ck + 1) * 4, :],
                    in_=wqkv_r[:, ck * 4 : (ck + 1) * 4, :],
                )
                xq = xts_pool.tile(
                    [HD, NT // 4, SB], BF16, tag="xts", bufs=4, name=f"xts_0_{ck}"
                )
                nc.sync.dma_start(out=xq, in_=xt_r[:, ck * 4 : (ck + 1) * 4, 0:SB])
                xts_tiles[(0, ck)] = xq
            for ck in range(4):
                xr = xts_pool.tile(
                    [HD, NT // 4, NSJ - 1, SB], BF16, tag="xrest", bufs=4,
                    name=f"xrest_{ck}",
                )
                nc.sync.dma_start(
                    out=xr, in_=xt_r[:, ck * 4 : (ck + 1) * 4, SB:S]
                )
                xrest_tiles[ck] = xr
            nc.gpsimd.dma_start(out=tri, in_=tri_d[:, :])
            nc.gpsimd.dma_start(out=ident, in_=ident_d[:, :])
            nc.gpsimd.dma_start(out=ones_sb, in_=ones_d[:, :])
            nc.scalar.dma_start(
                out=wo_sb, in_=wo_d[:, :].rearrange("(h p) d -> p h d", p=HD)
            )

            # deferred wo chains: stuffed into attention's exp-paced PE slack
            wo_queue = []

            def append_wo_block(sj):
                for stl in range(4):
                    st = sj * 4 + stl
                    t0 = st * HD
                    y_row = work.tile(
                        [HD, D], BF16, tag="ysb", bufs=2, name=f"yr_{st}"
                    )

                    def make_chain(st=st, t0=t0, y_row=y_row):
                        chains = []
                        for dj in range(NSJ):
                            def emit(dj=dj, st=st, t0=t0, y_row=y_row):
                                ps_y = ps.tile(
                                    [HD, SB], F32, tag="pp", bufs=2,
                                    name=f"ps_y_{st}_{dj}",
                                )
                                for hh in range(NH):
                                    nc.tensor.matmul(
                                        out=ps_y,
                                        lhsT=on_sb[:, hh, t0 : t0 + HD],
                                        rhs=wo_sb[:, hh, dj * SB : (dj + 1) * SB],
                                        start=(hh == 0),
                                        stop=(hh == NH - 1),
                                    )
                                nc.vector.tensor_copy(
                                    y_row[:, dj * SB : (dj + 1) * SB], ps_y
                                )
                                if dj == NSJ - 1:
                                    nc.sync.dma_start(
                                        out=y_d[t0 : t0 + HD, :], in_=y_row
                                    )
                            chains.append(emit)
                        return chains

                    wo_queue.extend(make_chain())

            def xq_ap(sj, dt):
                ck, sub = dt // 4, dt % 4
                if sj == 0:
                    return xts_tiles[(0, ck)][:, sub, :]
                return xrest_tiles[ck][:, sub, sj - 1, :]

            for sj in range(NSJ):
                s0 = sj * SB

                # ---- projection of x^T[:, s-block]: k, v, q0..q3 ----
                q_tiles = []
                for m in range(NM):
                    pp = ps.tile([HD, SB], F32, tag="pp", bufs=2, name=f"pp_{sj}_{m}")
                    for dt in range(NT):
                        nc.tensor.matmul(
                            out=pp,
                            lhsT=wqkv[:, dt, m * HD : (m + 1) * HD],
                            rhs=xq_ap(sj, dt),
                            start=(dt == 0),
                            stop=(dt == NT - 1),
                        )
                    if m == 1:
                        # v: psum holds v^T[hd, s-block]; transpose to v[t, hd]
                        vt = work.tile([HD, SB], BF16, tag="vt")
                        nc.scalar.copy(out=vt, in_=pp)
                        for qq in range(SB // HD):
                            pt = ps.tile(
                                [HD, HD], BF16, tag="pp", bufs=2, name=f"pt_{sj}_{qq}"
                            )
                            nc.tensor.transpose(pt, vt[:, qq * HD : (qq + 1) * HD], ident)
                            nc.scalar.copy(out=v_sb[:, sj * 4 + qq, :], in_=pt)
                    else:
                        # rope: rows 0:64 = even dims (xr), 64:128 = odd (xi)
                        # out_even = xr*c - xi*s ; out_odd = xr*s + xi*c
                        # muls on DVE (PSUM input side-steps the same-base-
                        # partition rule); combines on gpsimd (same-base SB)
                        if m == 0:
                            dst = kt[:, s0 : s0 + SB]
                        else:
                            dst = qk_pool.tile(
                                [HD, SB], BF16, tag="qk", bufs=8, name=f"q_{sj}_{m}"
                            )
                            q_tiles.append(dst)
                        c = cost[:, s0 : s0 + SB]
                        sn = sint[:, s0 : s0 + SB]
                        ta = work.tile([H2, SB], F32, tag="ropeA")
                        tb = work.tile([H2, SB], F32, tag="ropeB")
                        nc.vector.tensor_mul(out=ta, in0=pp[0:H2, :], in1=c)
                        nc.vector.tensor_mul(out=tb, in0=pp[H2:HD, :], in1=sn)
                        nc.gpsimd.tensor_sub(out=dst[0:H2, :], in0=ta, in1=tb)
                        tc2 = work.tile([H2, SB], F32, tag="ropeA")
                        td = work.tile([H2, SB], F32, tag="ropeB")
                        nc.vector.tensor_mul(out=tc2, in0=pp[0:H2, :], in1=sn)
                        nc.vector.tensor_mul(out=td, in0=pp[H2:HD, :], in1=c)
                        nc.gpsimd.tensor_add(out=dst[H2:HD, :], in0=tc2, in1=td)

                # ---- attention, software-pipelined over t-tiles ----
                # denominator: DVE accumulates es tiles in bf16 (exact enough;
                # validated ~4e-3 end-to-end), one ones-matmul colsum per head.
                # PE per tile is then scores+AV < exp, so deferred wo chains
                # are stuffed into the slack every STUFF_EVERY tiles.
                nt = 4 * sj + 4  # causal: t-tiles 0..nt-1
                LOOKAHEAD = 3
                STUFF_EVERY = 5
                tile_ctr = 0
                for h in range(NH):
                    qts = q_tiles[h]
                    ps_o = ps.tile([HD, SB], F32, tag="o", bufs=2, name=f"o_{sj}_{h}")
                    acc = es_pool.tile(
                        [HD, SB], BF16, tag="acc", bufs=2, name=f"acc_{sj}_{h}"
                    )

                    def emit_front(ti):
                        kdiag = ti - 4 * sj
                        c0 = max(0, kdiag) * HD  # first valid column (diag band)
                        ps_s = ps.tile(
                            [HD, SB], F32, tag="s", bufs=3, name=f"s_{sj}_{h}_{ti}"
                        )
                        nc.tensor.matmul(
                            out=ps_s[:, c0:SB],
                            lhsT=kt[:, ti * HD : (ti + 1) * HD],
                            rhs=qts[:, c0:SB],
                            start=True,
                            stop=True,
                        )
                        es = es_pool.tile(
                            [HD, SB], BF16, tag="es", bufs=6, name=f"es_{sj}_{h}_{ti}"
                        )
                        nc.scalar.activation(
                            out=es[:, c0:SB], in_=ps_s[:, c0:SB], func=Exp,
                            scale=float(SCALE),
                        )
                        if kdiag >= 0:
                            # triangular part: first HD valid columns
                            nc.gpsimd.tensor_mul(
                                out=es[:, c0 : c0 + HD],
                                in0=es[:, c0 : c0 + HD],
                                in1=tri,
                            )
                        if ti == 0:
                            nc.vector.tensor_copy(acc, es)
                        else:
                            nc.vector.tensor_add(
                                out=acc[:, c0:SB], in0=acc[:, c0:SB],
                                in1=es[:, c0:SB],
                            )
                        return (ti, es, c0)

                    def emit_back(item):
                        ti, es, c0 = item
                        nc.tensor.matmul(
                            out=ps_o[:, c0:SB],
                            lhsT=v_sb[:, ti, :],
                            rhs=es[:, c0:SB],
                            start=(ti == 0),
                            stop=(ti == nt - 1),
                        )

                    pend = []
                    for ti in range(nt):
                        pend.append(emit_front(ti))
                        if len(pend) > LOOKAHEAD:
                            emit_back(pend.pop(0))
                        tile_ctr += 1
                        if tile_ctr % STUFF_EVERY == 0 and wo_queue:
                            wo_queue.pop(0)()
                    while pend:
                        emit_back(pend.pop(0))

                    # den = colsum(acc), broadcast via all-ones stationary
                    ps_den = ps.tile(
                        [HD, SB], F32, tag="den", bufs=1, name=f"den_{sj}_{h}"
                    )
                    nc.tensor.matmul(
                        out=ps_den, lhsT=ones_sb, rhs=acc, start=True, stop=True
                    )
                    rb = work.tile([HD, SB], F32, tag="rb")
                    nc.vector.reciprocal_approx_fast(out=rb, in_=ps_den)
                    nc.vector.tensor_mul(
                        out=on_sb[:, h, s0 : s0 + SB], in0=ps_o, in1=rb
                    )

                # ---- wo chains for this s-block join the deferred queue ----
                append_wo_block(sj)
                # drain, keeping just enough to stuff the next block's slack
                keep = (4 * (4 * (sj + 1) + 4)) // STUFF_EVERY if sj < NSJ - 1 else 0
                while len(wo_queue) > keep:
                    wo_queue.pop(0)()
            while wo_queue:
                wo_queue.pop(0)()

    nc.compile()
    return nc


def _get_program():
    global _PROG
    if _PROG is None:
        _PROG = _build_program()
    return _PROG


def _make_in_maps(x, freqs_cos, freqs_sin, wq, wk, wv, wo):
    perm = np.concatenate([np.arange(0, HD, 2), np.arange(1, HD, 2)])  # even|odd

    costT = np.ascontiguousarray(np.asarray(freqs_cos, np.float32).T)  # [64, S]
    sintT = np.ascontiguousarray(np.asarray(freqs_sin, np.float32).T)

    tt = np.arange(HD)[:, None]
    ss = np.arange(HD)[None, :]
    tri = (tt <= ss).astype(BF)  # lower-tri in [t, s]: valid iff t <= s
    ident = np.eye(HD, dtype=BF)
    ones = np.ones((HD, HD), dtype=BF)

    # permute q/k head-dim columns so rope pairs land on partition halves
    def permute_heads(w, n_heads):
        w = np.asarray(w, np.float32).reshape(D, n_heads, HD)
        return w[:, :, perm].reshape(D, n_heads * HD)

    wq_p = permute_heads(wq, N_HEADS)
    wk_p = permute_heads(wk, N_KV_HEADS)
    wv_ = np.asarray(wv, np.float32)
    wo_ = np.asarray(wo, np.float32)
    x_ = np.asarray(x, np.float32)

    in_maps = []
    for c in range(8):
        b, g = divmod(c, 4)
        wqkv = np.concatenate(
            [
                wk_p[:, g * HD : (g + 1) * HD],
                wv_[:, g * HD : (g + 1) * HD],
                wq_p[:, g * NH * HD : (g + 1) * NH * HD],
            ],
            axis=1,
        )
        in_maps.append(
            {
                "xt": np.ascontiguousarray(x_[b].T).astype(BF),
                "wqkv": np.ascontiguousarray(wqkv).astype(BF),
                "wo": np.ascontiguousarray(
                    wo_[g * NH * HD : (g + 1) * NH * HD, :]
                ).astype(BF),
                "cost": costT,
                "sint": sintT,
                "tri": tri,
                "ident": ident,
                "ones": ones,
            }
        )
    return in_maps


def run(x, freqs_cos, freqs_sin, wq, wk, wv, wo, trace=False):
    from concourse.bass_utils import run_bass_kernel_spmd

    nc = _get_program()
    in_maps = _make_in_maps(x, freqs_cos, freqs_sin, wq, wk, wv, wo)
    res = run_bass_kernel_spmd(nc, in_maps, list(range(8)), trace=trace)
    out = np.empty((B, S, D), dtype=np.float32)
    for b in range(B):
        acc = res.results[b * 4]["y"].astype(np.float32)
        for g in range(1, 4):
            acc = acc + res.results[b * 4 + g]["y"].astype(np.float32)
        out[b] = acc
    return out, res


def kernel(x, freqs_cos, freqs_sin, wq, wk, wv, wo):
    out, _ = run(x, freqs_cos, freqs_sin, wq, wk, wv, wo, trace=False)
    return out
